# revision 10
# baseline (speedup 1.0000x reference)
"""Trainium2 Bass kernel for nn_CustomAttentionLayer (row-parallel attention).

Shards the K (query/row) dimension across 8 NeuronCores.  Each core gets a
256-row slice of the query-side tensors plus the full key-side tensors and
computes its slice of the output; the host concatenates the slices.

Per-core pipeline (all shapes per core; Kc=256 query rows, Kp=2048 keys,
D=512, H=32 MLP channels):

  u-phase   pairwise distances u[i,j]=|qc_i-kc_j| via a rank-4 fp32 matmul
            (dsq = |q|^2 - 2<q,k> + |k|^2) into PSUM + one ACT Sqrt pass.
            The 128-partition "u tile" interleaves, per 8-row chunk,
            [4 query rows, const 1.0, |k-s| row, pads] so the later H-MLP
            pre-activation can be built by a single matmul per group.
  bias      For each group of 4 query rows packed as (4 i x 32 c) on 128
            partitions: the H-MLP pre-activation (alpha_c*u + beta_ic +
            gamma_jc + b1) comes from ONE fp16 matmul against the u tile;
            ACT applies Relu into fp16 SBUF.  The G-MLP pre-activation is a
            single DVE tensor_scalar (row-replicated g_k + per-partition
            g_q bias, fused relu).  Both 32-channel contractions run on the
            PE with zero-padded (128,128) lhsT weights accumulating into a
            shared PSUM batch, giving R = relu(G+b2g)*(H+b2h) per i-tile.
  attn      S = (Q/sqrt(D)) @ K^T in fp16, logits = S + R, softmax row-wise
            (ACT Exp with fused accumulator), P^T via PE transposes,
            O = P^T-matmuls against K, fold 1/rowsum into the PSUM evac.
  LN/FFN    fp32 residual + layernorm (rstd via ACT Ln/Exp so no table
            switch), FFN as fp16 matmuls producing the hidden transposed
            (so no on-device transpose of the 2048-wide hidden), biases
            folded in as rank-1 accumulate-matmuls, final layernorm.
"""

import math
import sys

sys.path.insert(0, "/opt/trn_rl_repo")

import numpy as np

import concourse.bass as bass
import concourse.bacc as bacc
import concourse.mybir as mybir
import concourse.tile as tile
from concourse.bass_utils import run_bass_kernel_spmd

f32 = mybir.dt.float32
f16 = mybir.dt.float16
f32r = mybir.dt.float32r
AF = mybir.ActivationFunctionType
ALU = mybir.AluOpType
AX = mybir.AxisListType

K, D, H = 2048, 512, 32
NCORE = 8
KC = K // NCORE            # 256 query rows per core
NIT = KC // 128            # 2 i-tiles per core
NGRP = KC // 4             # 64 groups of 4 rows per core
NUT = KC // 64             # 4 u-tiles per core (64 rows each)
EPS = 1e-6
DSQ_EPS = 2e-3             # clamp guard under the sqrt (f32r dsq matmul)

_F16 = np.float16
_F32 = np.float32


def _host_prep(inputs):
    """Build the shared and per-core input maps (all cheap O(K) numpy)."""
    sensor = np.asarray(inputs["sensor_coords"], _F32)
    query = np.asarray(inputs["query"], _F32)
    key = np.asarray(inputs["key"], _F32)
    qc = np.asarray(inputs["q_cluster_centroids"], _F32)
    kc = np.asarray(inputs["k_cluster_centroids"], _F32)
    q_iso = np.asarray(inputs["q_iso"], _F32)
    k_iso = np.asarray(inputs["k_iso"], _F32)
    g_w1 = np.asarray(inputs["g_w1"], _F32)
    g_b1 = np.asarray(inputs["g_b1"], _F32)
    g_w2 = np.asarray(inputs["g_w2"], _F32)
    g_b2 = np.asarray(inputs["g_b2"], _F32)
    h_w1 = np.asarray(inputs["h_w1"], _F32)
    h_b1 = np.asarray(inputs["h_b1"], _F32)
    h_w2 = np.asarray(inputs["h_w2"], _F32)
    h_b2 = np.asarray(inputs["h_b2"], _F32)
    ffn_w1 = np.asarray(inputs["ffn_w1"], _F32)
    ffn_b1 = np.asarray(inputs["ffn_b1"], _F32)
    ffn_w2 = np.asarray(inputs["ffn_w2"], _F32)
    ffn_b2 = np.asarray(inputs["ffn_b2"], _F32)
    n1g = np.asarray(inputs["norm1_g"], _F32)
    n1b = np.asarray(inputs["norm1_b"], _F32)
    n2g = np.asarray(inputs["norm2_g"], _F32)
    n2b = np.asarray(inputs["norm2_b"], _F32)

    sx, sy = float(sensor[0]), float(sensor[1])
    kx, ky = kc[:, 0], kc[:, 1]

    shared = {}
    shared["kT16"] = np.ascontiguousarray(key.T).astype(_F16)          # (512, 2048)
    shared["kb16"] = key.astype(_F16)                                  # (2048, 512)
    shared["kside"] = np.stack(
        [kx, ky, kx * kx + ky * ky, np.ones(K, _F32)], axis=0
    ).astype(_F32)                                                     # (4, 2048)
    g_k = k_iso @ g_w1[12:]                                            # (2048, 32)
    shared["gkrep"] = np.tile(g_k.T, (4, 1)).astype(_F16)              # (128, 2048)
    shared["w116"] = ffn_w1.astype(_F16)                               # (512, 2048)
    shared["w216"] = ffn_w2.astype(_F16)                               # (2048, 512)
    shared["fb1"] = ffn_b1.reshape(1, 4 * D).astype(_F16)              # (1, 2048)
    shared["fb2"] = ffn_b2.reshape(1, D).astype(_F16)                  # (1, 512)
    shared["ones128"] = np.ones((1, 128), _F16)
    shared["ident"] = np.eye(128, dtype=_F16)

    # sliding-window contraction weights: win[p, 252 + p//32] = w2[p%32]
    pp = np.arange(128)
    w2win = np.zeros((128, 380), _F32)
    w2win[pp, 252 + pp // 32] = g_w2[pp % 32, 0]
    v2win = np.zeros((128, 380), _F32)
    v2win[pp, 252 + pp // 32] = h_w2[pp % 32, 0]
    shared["w2win"] = w2win.astype(_F16)
    shared["v2win"] = v2win.astype(_F16)

    shared["gb2c"] = np.full((128, 1), float(g_b2[0]), _F32)
    shared["dsqepsc"] = np.full((128, 1), DSQ_EPS, _F32)
    shared["lnepsc"] = np.full((128, 1), EPS, _F32)
    shared["hb2c"] = np.full((128, 1), float(h_b2[0]), _F32)
    shared["n1g"] = np.tile(n1g.reshape(1, D), (128, 1)).astype(_F32)
    shared["n1b"] = np.tile(n1b.reshape(1, D), (128, 1)).astype(_F32)
    shared["n2g"] = np.tile(n2g.reshape(1, D), (128, 1)).astype(_F32)
    shared["n2b"] = np.tile(n2b.reshape(1, D), (128, 1)).astype(_F32)

    in_maps = []
    for c in range(NCORE):
        r0 = c * KC
        qs = query[r0 : r0 + KC]
        qcs = qc[r0 : r0 + KC]
        qxs, qys = qcs[:, 0], qcs[:, 1]
        m = dict(shared)
        m["qT16"] = np.ascontiguousarray((qs / math.sqrt(D)).T).astype(_F16)  # (512, 256)
        m["qs"] = qs.astype(_F32)                                             # (256, 512)

        # dsq matmul lhsT: (4, NUT*128); 8-row chunks [4 rows, ones, kps, pad, pad]
        ulhs = np.zeros((4, NUT * 128), _F32)
        for ut in range(NUT):
            for p in range(128):
                ch, rr = p // 8, p % 8
                col = ut * 128 + p
                if rr < 4:
                    i = 64 * ut + 4 * ch + rr
                    ulhs[:, col] = [-2.0 * qxs[i], -2.0 * qys[i], 1.0,
                                    qxs[i] * qxs[i] + qys[i] * qys[i]]
                elif rr == 4:
                    ulhs[:, col] = [0.0, 0.0, 0.0, 1.0 - DSQ_EPS]
                elif rr == 5:
                    ulhs[:, col] = [-2.0 * sx, -2.0 * sy, 1.0,
                                    sx * sx + sy * sy]
                else:
                    ulhs[:, col] = [0.0, 0.0, 0.0, 1.0 - DSQ_EPS]
        m["ulhs"] = ulhs

        # H-MLP pre-activation lhsT per group: (128, NGRP*128) fp16.
        # Row 8ch+r (r<4) is u for i=4g+r; row 8ch+4 is 1.0; row 8ch+5 is kps.
        n_ks = np.sqrt((qxs - sx) ** 2 + (qys - sy) ** 2)              # (256,)
        mm_ = np.arange(128)
        a_of_m, c_of_m = mm_ // 32, mm_ % 32
        hblk = np.zeros((128, NGRP * 128), _F32)
        for g in range(NGRP):
            ch = g % 16
            base = 8 * ch
            cols = slice(g * 128, (g + 1) * 128)
            blk = np.zeros((128, 128), _F32)
            for r in range(4):
                blk[base + r, :] = h_w1[0, c_of_m] * (a_of_m == r)
            blk[base + 4, :] = n_ks[4 * g + a_of_m] * h_w1[1, c_of_m] + h_b1[c_of_m]
            blk[base + 5, :] = h_w1[2, c_of_m]
            hblk[:, cols] = blk
        m["hblk"] = hblk.astype(_F16)

        # G-MLP per-partition bias columns: (128, NGRP) fp32
        g_q = q_iso[r0 : r0 + KC] @ g_w1[:12]                          # (256, 32)
        gq = np.zeros((128, NGRP), _F32)
        for g in range(NGRP):
            gq[:, g] = g_q[4 * g + pp // 32, pp % 32] + g_b1[pp % 32]
        m["gqb"] = gq
        in_maps.append(m)
    return in_maps


_DECLS = [
    # (name, shape, dtype)
    ("qT16", [D, KC], f16),
    ("kT16", [D, K], f16),
    ("kb16", [K, D], f16),
    ("kside", [4, K], f32),
    ("ulhs", [4, NUT * 128], f32),
    ("hblk", [128, NGRP * 128], f16),
    ("gqb", [128, NGRP], f32),
    ("gkrep", [128, K], f16),
    ("w2win", [128, 380], f16),
    ("v2win", [128, 380], f16),
    ("gb2c", [128, 1], f32),
    ("dsqepsc", [128, 1], f32),
    ("lnepsc", [128, 1], f32),
    ("hb2c", [128, 1], f32),
    ("qs", [KC, D], f32),
    ("w116", [D, 4 * D], f16),
    ("w216", [4 * D, D], f16),
    ("fb1", [1, 4 * D], f16),
    ("fb2", [1, D], f16),
    ("ones128", [1, 128], f16),
    ("ident", [128, 128], f16),
    ("n1g", [128, D], f32),
    ("n1b", [128, D], f32),
    ("n2g", [128, D], f32),
    ("n2b", [128, D], f32),
]


def _layernorm_emit(nc, wp, z, gam, bet, out_f32, out_f16, epsc):
    """Emit LN of z (128, D) fp32 -> out tiles. rstd via Ln/Exp (exp table)."""
    nsum = wp.tile([128, 1], f32, tag="ln_s", bufs=6, name="nsum")
    nc.vector.tensor_reduce(nsum[:, :], z[:, :], axis=AX.X, op=ALU.add, negate=True)
    nmean = wp.tile([128, 1], f32, tag="ln_s", bufs=6, name="nmean")
    nc.vector.tensor_scalar_mul(nmean[:, :], nsum[:, :], 1.0 / D)
    xc = wp.tile([128, D], f32, tag="ln_xc", bufs=1, name="xc")
    nc.vector.tensor_scalar_add(xc[:, :], z[:, :], nmean[:, 0:1])
    sq = wp.tile([128, D], f32, tag="ln_t", bufs=2, name="sq")
    nc.vector.tensor_mul(sq[:, :], xc[:, :], xc[:, :])
    ssq = wp.tile([128, 1], f32, tag="ln_s", bufs=6, name="ssq")
    nc.vector.tensor_reduce(ssq[:, :], sq[:, :], axis=AX.X, op=ALU.add)
    lv = wp.tile([128, 1], f32, tag="ln_s", bufs=6, name="lv")
    nc.scalar.activation(lv[:, :], ssq[:, :], AF.Ln, bias=epsc[:, 0:1], scale=1.0 / D)
    rstd = wp.tile([128, 1], f32, tag="ln_s", bufs=6, name="rstd")
    nc.scalar.activation(rstd[:, :], lv[:, :], AF.Exp, scale=-0.5)
    xn = wp.tile([128, D], f32, tag="ln_t", bufs=2, name="xn")
    nc.vector.tensor_scalar_mul(xn[:, :], xc[:, :], rstd[:, 0:1])
    xg = wp.tile([128, D], f32, tag="ln_t", bufs=2, name="xg")
    nc.vector.tensor_mul(xg[:, :], xn[:, :], gam[:, :])
    nc.vector.tensor_add(out_f32[:, :], xg[:, :], bet[:, :])
    if out_f16 is not None:
        nc.vector.tensor_copy(out_f16[:, :], out_f32[:, :])


def _build():
    nc = bacc.Bacc(None, target_bir_lowering=False, debug=False)
    dh = {}
    for name, shape, dt_ in _DECLS:
        dh[name] = nc.declare_dram_parameter(name, shape, dt_, isOutput=False)
    out_d = nc.declare_dram_parameter("out", [KC, D], f32, isOutput=True)

    with tile.TileContext(nc) as tc:
        with tc.tile_pool(name="const", bufs=1) as cp, \
             tc.tile_pool(name="work", bufs=1) as wp:
            # ---- resident constants (bias-phase-critical first) ----
            kside_sb = cp.tile_from(dh["kside"][:, :])
            ulhs_sb = cp.tile_from(dh["ulhs"][:, :])
            hblk_sb = cp.tile_from(dh["hblk"][:, :])
            gqb_sb = cp.tile_from(dh["gqb"][:, :])
            gkrep_sb = cp.tile_from(dh["gkrep"][:, :])
            w2win_sb = cp.tile_from(dh["w2win"][:, :])
            v2win_sb = cp.tile_from(dh["v2win"][:, :])
            gb2c_sb = cp.tile_from(dh["gb2c"][:, :])
            dsqeps_sb = cp.tile_from(dh["dsqepsc"][:, :])
            lneps_sb = cp.tile_from(dh["lnepsc"][:, :])
            hb2c_sb = cp.tile_from(dh["hb2c"][:, :])

            # attention/FFN constants are traced after the bias-phase
            # constants so their (large) DMAs queue behind the critical ones
            qT_sb = cp.tile_from(dh["qT16"].ap().rearrange("(c p) i -> p c i", p=128))
            kT_sb = cp.tile_from(dh["kT16"].ap().rearrange("(c p) j -> p c j", p=128))
            kb_sb = cp.tile_from(dh["kb16"].ap().rearrange("(t p) d -> p t d", p=128))
            ident_sb = cp.tile_from(dh["ident"][:, :])
            w1_sb = cp.tile_from(dh["w116"].ap().rearrange("(c p) h -> p c h", p=128))
            w2h_sb = cp.tile_from(dh["w216"].ap().rearrange("(t p) d -> p t d", p=128))
            fb1_sb = cp.tile_from(dh["fb1"][:, :])
            fb2_sb = cp.tile_from(dh["fb2"][:, :])
            ones_sb = cp.tile_from(dh["ones128"][:, :])
            n1g_sb = cp.tile_from(dh["n1g"][:, :])
            n1b_sb = cp.tile_from(dh["n1b"][:, :])
            n2g_sb = cp.tile_from(dh["n2g"][:, :])
            n2b_sb = cp.tile_from(dh["n2b"][:, :])

            # ---- u phase: dsq matmul (f32r) + sqrt ----
            u16 = []
            with tc.tile_pool(name="psU", bufs=2, space="PSUM") as pU:
                for ut in range(NUT):
                    psu = pU.tile([128, K], f32, tag="u", name="psu")
                    for nn in range(4):
                        nc.tensor.matmul(
                            psu[:, nn * 512 : (nn + 1) * 512],
                            ulhs_sb[:, ut * 128 : (ut + 1) * 128],
                            kside_sb[:, nn * 512 : (nn + 1) * 512],
                            start=True, stop=True,
                        )
                    ut_sb = cp.tile([128, K], f16, name=f"u16_{ut}")
                    nc.scalar.activation(ut_sb[:, :], psu[:, :], AF.Sqrt,
                                         bias=dsqeps_sb[:, 0:1], scale=1.0)
                    u16.append(ut_sb)

            for it in range(NIT):
                # ================= bias phase =================
                R_sb = wp.tile([128, K], f16, tag="R", bufs=1, name="R_sb")
                with tc.tile_pool(name=f"psB{it}", bufs=1, space="PSUM") as pB:
                    for jh in range(2):
                        js = slice(jh * 1024, (jh + 1) * 1024)
                        psG = pB.tile([128, 1024], f32, tag="gb", name="psG")
                        psH = pB.tile([128, 1024], f32, tag="hb", name="psH")
                        for k in range(32):
                            g = 32 * it + k
                            ut = g // 16
                            # H pre-activation: one matmul vs the u tile
                            pshp = pB.tile([128, 1024], f32, tag="hp", bufs=2,
                                           name="pshp")
                            for hh in range(2):
                                nc.tensor.matmul(
                                    pshp[:, hh * 512 : (hh + 1) * 512],
                                    hblk_sb[:, g * 128 : (g + 1) * 128],
                                    u16[ut][:, jh * 1024 + hh * 512 :
                                            jh * 1024 + (hh + 1) * 512],
                                    start=True, stop=True,
                                )
                            hact = wp.tile([128, 1024], f16, tag="hact", bufs=4,
                                           name="hact")
                            if k % 3 == 2:
                                nc.vector.tensor_scalar(hact[:, :], pshp[:, :],
                                                        0.0, None, op0=ALU.max)
                            else:
                                nc.scalar.activation(hact[:, :], pshp[:, :], AF.Relu)
                            gact = wp.tile([128, 1024], f16, tag="gact", bufs=3,
                                           name="gact")
                            nc.vector.tensor_scalar(
                                gact[:, :], gkrep_sb[:, js], gqb_sb[:, g : g + 1],
                                0.0, op0=ALU.add, op1=ALU.max)
                            for hh in range(2):
                                hsl = slice(hh * 512, (hh + 1) * 512)
                                nc.tensor.matmul(
                                    psG[:, hsl],
                                    w2win_sb[:, 252 - 4 * k : 380 - 4 * k],
                                    gact[:, hsl], start=(k == 0), stop=(k == 31))
                                nc.tensor.matmul(
                                    psH[:, hsl],
                                    v2win_sb[:, 252 - 4 * k : 380 - 4 * k],
                                    hact[:, hsl], start=(k == 0), stop=(k == 31))
                        tmpG = wp.tile([128, 1024], f16, tag="tmpG", bufs=1, name="tmpG")
                        nc.vector.tensor_scalar(tmpG[:, :], psG[:, :],
                                                gb2c_sb[:, 0:1], 0.0,
                                                op0=ALU.add, op1=ALU.max)
                        tmpH = wp.tile([128, 1024], f16, tag="tmpH", bufs=1, name="tmpH")
                        nc.vector.tensor_scalar_add(tmpH[:, :], psH[:, :],
                                                    hb2c_sb[:, 0:1])
                        nc.vector.tensor_mul(R_sb[:, js], tmpG[:, :], tmpH[:, :])

                # ================= attention phase =================
                with tc.tile_pool(name=f"psA{it}", bufs=1, space="PSUM") as pA:
                    psS = pA.tile([128, K], f32, tag="big", name="psS")
                    for jj in range(4):
                        for cc in range(4):
                            nc.tensor.matmul(
                                psS[:, jj * 512 : (jj + 1) * 512],
                                qT_sb[:, cc, it * 128 : (it + 1) * 128],
                                kT_sb[:, cc, jj * 512 : (jj + 1) * 512],
                                start=(cc == 0), stop=(cc == 3))
                    L_sb = wp.tile([128, K], f16, tag="L", bufs=1, name="L_sb")
                    nc.vector.tensor_tensor(L_sb[:, :], psS[:, :], R_sb[:, :],
                                            op=ALU.add)
                    negmax = wp.tile([128, 1], f32, tag="sm_s", bufs=4, name="negmax")
                    nc.vector.tensor_reduce(negmax[:, :], L_sb[:, :], axis=AX.X,
                                            op=ALU.max, negate=True)
                    P_sb = wp.tile([128, K], f16, tag="P", bufs=1, name="P_sb")
                    rsum = wp.tile([128, 1], f32, tag="sm_s", bufs=4, name="rsum")
                    nc.scalar.activation(P_sb[:, :], L_sb[:, :], AF.Exp,
                                         bias=negmax[:, 0:1], scale=1.0,
                                         accum_out=rsum[:, 0:1])
                    rrow = wp.tile([128, 1], f32, tag="sm_s", bufs=4, name="rrow")
                    nc.vector.reciprocal(rrow[:, :], rsum[:, :])

                    psPT = pA.tile([128, K], f16, tag="pt", name="psPT")
                    for jt in range(16):
                        nc.tensor.transpose(
                            psPT[:, jt * 128 : (jt + 1) * 128],
                            P_sb[:, jt * 128 : (jt + 1) * 128], ident_sb[:, :])
                    PT_sb = wp.tile([128, K], f16, tag="PT", bufs=1, name="PT_sb")
                    nc.scalar.copy(PT_sb[:, :], psPT[:, :])

                    psO = pA.tile([128, D], f32, tag="small", name="psO")
                    for jt in range(16):
                        nc.tensor.matmul(
                            psO[:, :],
                            PT_sb[:, jt * 128 : (jt + 1) * 128],
                            kb_sb[:, jt, :],
                            start=(jt == 0), stop=(jt == 15))
                    O_sb = wp.tile([128, D], f32, tag="O", bufs=1, name="O_sb")
                    nc.vector.tensor_scalar_mul(O_sb[:, :], psO[:, :], rrow[:, 0:1])

                # residual + LN1 (fp32)
                q_sb = wp.tile([128, D], f32, tag="q", bufs=2, name="q_sb")
                nc.sync.dma_start(q_sb[:, :], dh["qs"][it * 128 : (it + 1) * 128, :])
                z1 = wp.tile([128, D], f32, tag="z", bufs=2, name="z1")
                nc.vector.tensor_add(z1[:, :], O_sb[:, :], q_sb[:, :])
                x1 = wp.tile([128, D], f32, tag="x1", bufs=1, name="x1")
                x1h = wp.tile([128, D], f16, tag="x1h", bufs=1, name="x1h")
                _layernorm_emit(nc, wp, z1, n1g_sb, n1b_sb, x1, x1h, lneps_sb)

                # ================= FFN phase =================
                with tc.tile_pool(name=f"psF{it}", bufs=1, space="PSUM") as pF:
                    psxT = pF.tile([128, D], f16, tag="xtp", name="psxT")
                    for ccc in range(4):
                        nc.tensor.transpose(
                            psxT[:, ccc * 128 : (ccc + 1) * 128],
                            x1h[:, ccc * 128 : (ccc + 1) * 128], ident_sb[:, :])
                    xT_sb = wp.tile([128, D], f16, tag="xT", bufs=1, name="xT_sb")
                    nc.vector.tensor_copy(xT_sb[:, :], psxT[:, :])

                    psF = pF.tile([128, 4 * D], f32, tag="fbig", name="psF")
                    for hc in range(16):
                        hs = slice(hc * 128, (hc + 1) * 128)
                        for ccc in range(4):
                            nc.tensor.matmul(
                                psF[:, hs],
                                w1_sb[:, ccc, hc * 128 : (hc + 1) * 128],
                                xT_sb[:, ccc * 128 : (ccc + 1) * 128],
                                start=(ccc == 0), stop=False)
                        nc.tensor.matmul(
                            psF[:, hs], fb1_sb[0:1, hs], ones_sb[0:1, :],
                            start=False, stop=True)
                    hT_sb = wp.tile([128, 4 * D], f16, tag="hT", bufs=1, name="hT_sb")
                    nc.scalar.activation(hT_sb[:, :], psF[:, :], AF.Relu)

                    psO2 = pF.tile([128, D], f32, tag="small", bufs=3, name="psO2")
                    for hc in range(16):
                        nc.tensor.matmul(
                            psO2[:, :],
                            hT_sb[:, hc * 128 : (hc + 1) * 128],
                            w2h_sb[:, hc, :],
                            start=(hc == 0), stop=False)
                    nc.tensor.matmul(psO2[:, :], ones_sb[0:1, :], fb2_sb[0:1, :],
                                     start=False, stop=True)
                    z2 = wp.tile([128, D], f32, tag="z", bufs=2, name="z2")
                    nc.vector.tensor_tensor(z2[:, :], psO2[:, :], x1[:, :],
                                            op=ALU.add)

                y = wp.tile([128, D], f32, tag="y", bufs=1, name="y")
                _layernorm_emit(nc, wp, z2, n2g_sb, n2b_sb, y, None, lneps_sb)
                nc.sync.dma_start(out_d[it * 128 : (it + 1) * 128, :], y[:, :])

    nc.compile()
    return nc


_BUILT = {}


def _get_built():
    if "nc" not in _BUILT:
        _BUILT["nc"] = _build()
    return _BUILT["nc"]


def kernel(**inputs):
    in_maps = _host_prep(inputs)
    nc = _get_built()
    res = run_bass_kernel_spmd(nc, in_maps, core_ids=list(range(NCORE)))
    return np.concatenate([res.results[c]["out"] for c in range(NCORE)], axis=0)


if __name__ == "__main__":
    import reference

    inputs = {k: np.asarray(v) for k, v in reference.setup_inputs().items()}
    expected = np.asarray(reference.reference(**inputs))
    actual = kernel(**inputs)
    err = np.abs(actual - expected).max() / np.abs(expected).max()
    print("Relative error:", err)
